# revision 5
# baseline (speedup 1.0000x reference)
"""Cumulative (causal) normalization kernel for TRN2, 8 NeuronCores.

x: [32, 512, 4000] f32.  out = (x - cum_mean) / sqrt(cum_var + eps), cumsum
along frames.  Data-parallel: rows = batch*bins flattened -> 16384 rows,
2048 rows per core.  Per 128-row x 2000-frame half-tile:

  xsq  = x^2                                  (ACT Square)
  s1   = cumsum(x)                            (DVE scan)
  s2e  = cumsum(xsq) + n*eps                  (DVE scan, data1=eps)
  t1   = x * n                                (Pool TT)
  num  = t1 - s1                              (DVE TT)
  t2   = s2e * n                              (DVE/Pool TT)
  t3   = s1^2                                 (ACT Square)
  W    = t2 - t3        (= n^2*(var+eps))     (DVE TT)
  r    = 1/sqrt(|W|)                          (ACT Abs_reciprocal_sqrt)
  out  = num * r                              (DVE TT)

The frame-chain is split across the two half-tiles by seeding the second
half's scans with the first half's final prefix values (scan initial=AP).
"""

import numpy as np

EPS = 1e-4
B, NBINS, F = 32, 512, 4000
P = 128
FD = 2000          # frames per half-tile
NCORES = 8
ROWS = B * NBINS               # 16384
ROWS_PER_CORE = ROWS // NCORES  # 2048
NT = ROWS_PER_CORE // P         # 16 row-tiles per core

_CACHE = {}


def _build():
    import concourse.bacc as bacc
    import concourse.mybir as mybir
    import concourse.tile as tile

    f32 = mybir.dt.float32
    nc = bacc.Bacc()

    x_d = nc.dram_tensor("x", [ROWS_PER_CORE, F], f32, kind="ExternalInput")
    n_d = nc.dram_tensor("nmul", [P, F], f32, kind="ExternalInput")
    o_d = nc.dram_tensor("out", [ROWS_PER_CORE, F], f32, kind="ExternalOutput")

    add = mybir.AluOpType.add
    byp = mybir.AluOpType.bypass
    SQ = mybir.ActivationFunctionType.Square
    ARS = mybir.ActivationFunctionType.Abs_reciprocal_sqrt

    with tile.TileContext(nc) as tc:
        with (
            tc.tile_pool(name="cst", bufs=1) as cst,
            tc.tile_pool(name="io", bufs=2) as io,
            tc.tile_pool(name="io2", bufs=2) as io2,
            tc.tile_pool(name="wk", bufs=2) as wk,
            tc.tile_pool(name="wx", bufs=1) as wx,
        ):
            nmul = cst.tile([P, F], f32)
            nc.sync.dma_start(out=nmul, in_=n_d[:, :])
            epst = cst.tile([P, FD], f32)
            nc.vector.memset(epst, EPS)

            for it in range(NT):
                r0 = it * P
                x_t = io.tile([P, F], f32, tag="x")
                nc.sync.dma_start(out=x_t, in_=x_d[r0:r0 + P, :])
                out_t = io2.tile([P, F], f32, tag="o")

                t1f = wk.tile([P, F], f32, tag="t1f")
                nc.gpsimd.tensor_mul(t1f, x_t, nmul)

                prev_s1 = None
                prev_s2e = None
                for h in range(2):
                    lo = h * FD
                    hi = lo + FD
                    xs = x_t[:, lo:hi]
                    ns = nmul[:, lo:hi]

                    xsq = wx.tile([P, FD], f32, tag="xsq")
                    nc.scalar.activation(xsq, xs, SQ)

                    s1 = wk.tile([P, FD], f32, tag="s1")
                    nc.vector.tensor_tensor_scan(
                        out=s1, data0=xs, data1=xs,
                        initial=(0.0 if h == 0 else prev_s1[:, FD - 1:FD]),
                        op0=add, op1=byp)

                    s2e = wk.tile([P, FD], f32, tag="s2e")
                    nc.vector.tensor_tensor_scan(
                        out=s2e, data0=xsq, data1=epst,
                        initial=(0.0 if h == 0 else prev_s2e[:, FD - 1:FD]),
                        op0=add, op1=add)
                    prev_s1, prev_s2e = s1, s2e

                    num = wx.tile([P, FD], f32, tag="num")
                    nc.vector.tensor_sub(num, t1f[:, lo:hi], s1)

                    t2 = wk.tile([P, FD], f32, tag="t2")
                    if (it + h) % 2 == 0:
                        nc.gpsimd.tensor_mul(t2, s2e, ns)
                    else:
                        nc.vector.tensor_mul(t2, s2e, ns)

                    t3 = wx.tile([P, FD], f32, tag="t3")
                    nc.scalar.activation(t3, s1, SQ)

                    w = wx.tile([P, FD], f32, tag="w")
                    nc.vector.tensor_sub(w, t2, t3)

                    r = wx.tile([P, FD], f32, tag="r")
                    nc.scalar.activation(r, w, ARS)

                    nc.vector.tensor_mul(out_t[:, lo:hi], num, r)

                nc.sync.dma_start(out=o_d[r0:r0 + P, :], in_=out_t)

    nc.finalize()
    return nc


def kernel(x: np.ndarray) -> np.ndarray:
    from concourse import bass_utils

    assert x.shape == (B, NBINS, F) and x.dtype == np.float32
    if "nc" not in _CACHE:
        _CACHE["nc"] = _build()
    nc = _CACHE["nc"]

    nmul = np.broadcast_to(
        np.arange(1, F + 1, dtype=np.float32)[None, :], (P, F)
    ).copy()

    xf = np.ascontiguousarray(x.reshape(ROWS, F))
    in_maps = [
        {"x": xf[c * ROWS_PER_CORE:(c + 1) * ROWS_PER_CORE], "nmul": nmul}
        for c in range(NCORES)
    ]
    res = bass_utils.run_bass_kernel_spmd(nc, in_maps, core_ids=list(range(NCORES)))
    out = np.concatenate([r["out"] for r in res.results], axis=0)
    return out.reshape(B, NBINS, F)


# revision 6
# speedup vs baseline: 1.1591x; 1.1591x over previous
"""Cumulative (causal) normalization kernel for TRN2, 8 NeuronCores.

x: [32, 512, 4000] f32.  out = (x - cum_mean) / sqrt(cum_var + eps), cumsum
along frames.  Data-parallel: rows = batch*bins flattened -> 16384 rows,
2048 rows per core.  Per 128-row x 2000-frame half-tile:

  xsq  = x^2                                  (ACT Square)
  s1   = cumsum(x)                            (DVE scan)
  s2e  = cumsum(xsq) + n*eps                  (DVE scan, data1=eps)
  t1   = x * n                                (Pool TT)
  num  = t1 - s1                              (DVE TT)
  t2   = s2e * n                              (DVE/Pool TT)
  t3   = s1^2                                 (ACT Square)
  W    = t2 - t3        (= n^2*(var+eps))     (DVE TT)
  r    = 1/sqrt(|W|)                          (ACT Abs_reciprocal_sqrt)
  out  = num * r                              (DVE TT)

The frame-chain is split across the two half-tiles by seeding the second
half's scans with the first half's final prefix values (scan initial=AP).
"""

import numpy as np

EPS = 1e-4
B, NBINS, F = 32, 512, 4000
P = 128
FD = 2000          # frames per half-tile
NCORES = 8
ROWS = B * NBINS               # 16384
ROWS_PER_CORE = ROWS // NCORES  # 2048
NT = ROWS_PER_CORE // P         # 16 row-tiles per core

_CACHE = {}


def _build():
    import concourse.bacc as bacc
    import concourse.mybir as mybir
    import concourse.tile as tile

    f32 = mybir.dt.float32
    nc = bacc.Bacc()

    x_d = nc.dram_tensor("x", [ROWS_PER_CORE, F], f32, kind="ExternalInput")
    n_d = nc.dram_tensor("nmul", [P, F], f32, kind="ExternalInput")
    o_d = nc.dram_tensor("out", [ROWS_PER_CORE, F], f32, kind="ExternalOutput")

    add = mybir.AluOpType.add
    byp = mybir.AluOpType.bypass
    SQ = mybir.ActivationFunctionType.Square
    ARS = mybir.ActivationFunctionType.Abs_reciprocal_sqrt

    with tile.TileContext(nc) as tc:
        with (
            tc.tile_pool(name="cst", bufs=1) as cst,
            tc.tile_pool(name="io", bufs=2) as io,
            tc.tile_pool(name="io2", bufs=2) as io2,
            tc.tile_pool(name="wk", bufs=2) as wk,
            tc.tile_pool(name="wx", bufs=1) as wx,
        ):
            nmul = cst.tile([P, F], f32)
            nc.sync.dma_start(out=nmul, in_=n_d[:, :])
            epst = cst.tile([P, FD], f32)
            nc.vector.memset(epst, EPS)

            for it in range(NT):
                r0 = it * P
                x_t = io.tile([P, F], f32, tag="x")
                nc.sync.dma_start(out=x_t, in_=x_d[r0:r0 + P, :])
                out_t = io2.tile([P, F], f32, tag="o")

                t1f = wk.tile([P, F], f32, tag="t1f")
                nc.vector.tensor_mul(t1f, x_t, nmul)

                prev_s1 = None
                prev_s2e = None
                for h in range(2):
                    lo = h * FD
                    hi = lo + FD
                    xs = x_t[:, lo:hi]
                    ns = nmul[:, lo:hi]

                    xsq = wx.tile([P, FD], f32, tag="xsq")
                    nc.scalar.activation(xsq, xs, SQ)

                    s1 = wk.tile([P, FD], f32, tag="s1")
                    nc.vector.tensor_tensor_scan(
                        out=s1, data0=xs, data1=xs,
                        initial=(0.0 if h == 0 else prev_s1[:, FD - 1:FD]),
                        op0=add, op1=byp)

                    s2e = wk.tile([P, FD], f32, tag="s2e")
                    nc.vector.tensor_tensor_scan(
                        out=s2e, data0=xsq, data1=epst,
                        initial=(0.0 if h == 0 else prev_s2e[:, FD - 1:FD]),
                        op0=add, op1=add)
                    prev_s1, prev_s2e = s1, s2e

                    num = wx.tile([P, FD], f32, tag="num")
                    nc.vector.tensor_sub(num, t1f[:, lo:hi], s1)

                    t2 = wk.tile([P, FD], f32, tag="t2")
                    nc.vector.tensor_mul(t2, s2e, ns)

                    t3 = wx.tile([P, FD], f32, tag="t3")
                    nc.scalar.activation(t3, s1, SQ)

                    w = wx.tile([P, FD], f32, tag="w")
                    nc.vector.tensor_sub(w, t2, t3)

                    r = wx.tile([P, FD], f32, tag="r")
                    nc.scalar.activation(r, w, ARS)

                    nc.vector.tensor_mul(out_t[:, lo:hi], num, r)

                nc.sync.dma_start(out=o_d[r0:r0 + P, :], in_=out_t)

    nc.finalize()
    return nc


def kernel(x: np.ndarray) -> np.ndarray:
    from concourse import bass_utils

    assert x.shape == (B, NBINS, F) and x.dtype == np.float32
    if "nc" not in _CACHE:
        _CACHE["nc"] = _build()
    nc = _CACHE["nc"]

    nmul = np.broadcast_to(
        np.arange(1, F + 1, dtype=np.float32)[None, :], (P, F)
    ).copy()

    xf = np.ascontiguousarray(x.reshape(ROWS, F))
    in_maps = [
        {"x": xf[c * ROWS_PER_CORE:(c + 1) * ROWS_PER_CORE], "nmul": nmul}
        for c in range(NCORES)
    ]
    res = bass_utils.run_bass_kernel_spmd(nc, in_maps, core_ids=list(range(NCORES)))
    out = np.concatenate([r["out"] for r in res.results], axis=0)
    return out.reshape(B, NBINS, F)
